# revision 36
# baseline (speedup 1.0000x reference)
"""Trainium2 Bass kernel for two-stage pooled-query attention.

Problem (hardcoded):
    B=32, N=577, C=1024, H=16 heads, d=64, pooled queries 8x8 (3x3 mean over
    24x24 grid of non-cls tokens).
    qkv = X @ W_qkv.T ; pool Xq -> Qp ; s1 = softmax(Qp*s @ K^T) @ V ;
    s2 = softmax(Xq*s @ Qp^T) @ s1 ; out = s2 @ W_proj.T + b_proj

Strategy: pure data-parallel over batch across 8 NeuronCores (4 batches per
core, no collectives).

The two big GEMMs (QKV and output projection, ~75% of matmul cycles) run as
3-term fp8e4 DoubleRow matmuls: A@B ~ Ahi@Bhi + Alo@Bhi + (Ahi/16)@(16*Blo),
with K-chunk pairs packed into each DoubleRow instruction (effective K=256
per instruction at 0.5 cycles/row -> 4x bf16 throughput per K=128 product,
so the 3-term scheme runs at 0.75x bf16 cost with ~bf16 accuracy). X and the
weights arrive pre-split/interleaved from the host (layout prep); the
attention output is split on-chip during the PSUM evict, pre-scaled x16 (via
a 1/16 ones-column in V that scales Qd) so the residual stays out of fp8's
subnormal-flush range; the host divides the output by 16.

Attention middle stays bf16 (contractions are K<=128 there, DoubleRow does
not apply). Stage 1 is computed transposed relative to the baseline:
scores E1T[n, 2q] = exp(K^T Qp) directly (no PE transpose of A1), and the
softmax denominator comes from the ones-column in V, so Qd = E1T^T @ [V | 1]
yields numerator and denominator in one accumulation.

The whole thing is software-pipelined across batches: batch b+1's QKV/V
chunk-groups are emitted as PE filler interleaved into batch b's attention
middle (qkT/V/X are double-buffered), so the PE never drains during the
dependency-heavy softmax stretches.
"""

import os
import sys

import numpy as np

sys.path.insert(0, "/opt/trn_rl_repo")

import ml_dtypes  # noqa: E402

import concourse.tile as tile  # noqa: E402
from concourse import bacc, mybir  # noqa: E402
from concourse.bass_utils import run_bass_kernel_spmd  # noqa: E402
from concourse.masks import make_identity  # noqa: E402

B, N, C = 32, 577, 1024
H, D = 16, 64
SCALE = D ** -0.5
N_CORES = 8
NB = B // N_CORES  # batches per core

BF16 = mybir.dt.bfloat16
FP8 = mybir.dt.float8e4
F32 = mybir.dt.float32
DR = mybir.MatmulPerfMode.DoubleRow
NP8 = ml_dtypes.float8_e4m3

# token chunks of 577 = 4*128 + 65
TOK = [(0, 128), (128, 128), (256, 128), (384, 128), (512, 65)]
# free-dim chunks of 577 for wide matmuls / psum banks
NF = [(0, 320), (320, 257)]
EXP = mybir.ActivationFunctionType.Exp
SUB = mybir.AluOpType.subtract


def build_program(nb: int = NB):
    nc = bacc.Bacc("TRN2", target_bir_lowering=False, debug=False)

    # X pre-transposed [c, n], zero-padded to 640 tokens, split hi/lo/hi16
    # fp8 and K-pair interleaved: [pc, part, plane, tok].
    xhi_d = nc.dram_tensor("xhi", [nb, 128, 4, 2, 640], FP8, kind="ExternalInput")
    xlo_d = nc.dram_tensor("xlo", [nb, 128, 4, 2, 640], FP8, kind="ExternalInput")
    xh16_d = nc.dram_tensor("xh16", [nb, 128, 4, 2, 640], FP8, kind="ExternalInput")
    wqh_d = nc.dram_tensor("wqh", [128, 4, 2, 3 * C], FP8, kind="ExternalInput")
    wql_d = nc.dram_tensor("wql", [128, 4, 2, 3 * C], FP8, kind="ExternalInput")
    wph_d = nc.dram_tensor("wph", [128, 4, 2, C], FP8, kind="ExternalInput")
    wpl_d = nc.dram_tensor("wpl", [128, 4, 2, C], FP8, kind="ExternalInput")
    wbias_d = nc.dram_tensor("wbias", [1, C], BF16, kind="ExternalInput")
    out_d = nc.dram_tensor("out", [nb, N, C], BF16, kind="ExternalOutput")

    with tile.TileContext(nc) as tc:
        const_pool = tc.alloc_tile_pool(name="const", bufs=1)
        w_pool = tc.alloc_tile_pool(name="w", bufs=1)
        sb = tc.alloc_tile_pool(name="sb", bufs=2)
        ps_big = tc.alloc_tile_pool(name="ps_big", bufs=5, space="PSUM")
        ps_small = tc.alloc_tile_pool(name="ps_small", bufs=3, space="PSUM")

        ident = const_pool.tile([128, 128], BF16, tag="ident")
        make_identity(nc, ident[:])
        ones = const_pool.tile([1, 128], BF16, tag="ones")
        nc.gpsimd.memset(ones[:], 1.0)

        def dma_x_grp(b, grp):
            """DMA one of the three X tensors (0=hi, 1=lo, 2=hi16)."""
            tag, dram = [("xh", xhi_d), ("xl", xlo_d), ("x16", xh16_d)][grp]
            t = sb.tile([128, 4, 2, 640], FP8, tag=tag, bufs=2, name=tag)
            nc.sync.dma_start(t[:], dram[b])
            return t

        def dma_x(b):
            return tuple(dma_x_grp(b, g) for g in range(3))

        # weights stream in 512-col chunks interleaved with batch-0's X in
        # the order the QKV gemm consumes them (term-major: hi*hi first).
        WQH = w_pool.tile([128, 4, 2, 3 * C], FP8, tag="wqh", name="wqh")
        # WQL holds only the V columns: the q/k channels use the 2-term
        # product, so the lo-correction weights for them are never read
        WQL = w_pool.tile([128, 4, 2, C], FP8, tag="wql", name="wql")

        def dma_wq(which, blk):
            cs = slice(512 * blk, 512 * (blk + 1))
            if which == 0:
                nc.sync.dma_start(WQH[:, :, :, cs], wqh_d[:, :, :, cs])
            else:
                nc.sync.dma_start(
                    WQL[:, :, :, cs],
                    wql_d[:, :, :, 2048 + 512 * blk : 2048 + 512 * (blk + 1)],
                )

        repeat = int(os.environ.get("KERNEL_REPEAT", "1"))
        order = [bb for _ in range(repeat) for bb in range(nb)]

        def dma_x_grp_split(b, grp):
            tag, dram = [("xh", xhi_d), ("xl", xlo_d), ("x16", xh16_d)][grp]
            t = sb.tile([128, 4, 2, 640], FP8, tag=tag, bufs=2, name=tag)
            nc.sync.dma_start(t[:, 0:2], dram[b, :, 0:2])
            return t

        def finish_x_grp(t, b, grp):
            dram = [xhi_d, xlo_d, xh16_d][grp]
            nc.sync.dma_start(t[:, 2:4], dram[b, :, 2:4])

        XH0 = dma_x_grp_split(order[0], 0)
        nc.sync.dma_start(WQH[:, 0:2, :, 0:512], wqh_d[:, 0:2, :, 0:512])
        finish_x_grp(XH0, order[0], 0)
        nc.sync.dma_start(WQH[:, 2:4, :, 0:512], wqh_d[:, 2:4, :, 0:512])
        XL0 = dma_x_grp(order[0], 1)
        for blk in range(1, 4):
            dma_wq(0, blk)
        X160 = dma_x_grp(order[0], 2)
        X0 = (XH0, XL0, X160)
        dma_wq(0, 4)
        dma_wq(0, 5)
        dma_wq(1, 0)
        dma_wq(1, 1)
        # batch 1's X goes out before the proj weights: the pipelined filler
        # needs it early, the proj weights only by the first proj phase
        pre_x = {}
        if len(order) > 1:
            pre_x[1] = dma_x(order[1])
        WPH = w_pool.tile([128, 4, 2, C], FP8, tag="wph", name="wph")
        WPL = w_pool.tile([128, 4, 2, C], FP8, tag="wpl", name="wpl")
        nc.sync.dma_start(WPH[:], wph_d[:])
        nc.sync.dma_start(WPL[:], wpl_d[:])
        wb = w_pool.tile([1, C], BF16, tag="wb")
        nc.sync.dma_start(wb[:], wbias_d[:])

        # ------------------------------------------------------------------
        # per-batch state + emitters
        # ------------------------------------------------------------------
        class St:
            pass

        def make_state(b, X3):
            S = St()
            S.b = b
            S.XH, S.XL, S.X16 = X3
            S.qkT = [None] * 16
            S.QpBD = [None] * 8
            S.V = [None] * 5
            S.QdBD = [None] * 8
            S.A2n = [[None, None] for _ in range(5)]
            S.s1e = {}
            S.a2t = {}
            S.OTH = [sb.tile([128, 2, 640], FP8, tag=f"oth{pq}", bufs=1,
                             name=f"oth{pq}") for pq in range(4)]
            S.OTL = [sb.tile([128, 2, 640], FP8, tag=f"otl{pq}", bufs=1,
                             name=f"otl{pq}") for pq in range(4)]
            S.OT16 = [sb.tile([128, 2, 640], FP8, tag=f"ot16{pq}", bufs=1,
                              name=f"ot16{pq}") for pq in range(4)]
            return S

        def pool_chunk(S, j):
            qsum = sb.tile([128, 64], F32, tag="qsum", bufs=3)
            view = S.qkT[j][:, 0:576].rearrange(
                "p (pr dr pc dc) -> p pr pc dr dc", pr=8, dr=3, pc=8, dc=3
            )
            nc.vector.reduce_sum(qsum[:], view, axis=mybir.AxisListType.XY)
            qp = sb.tile([128, 128], BF16, tag=f"qp{j}", bufs=2, name=f"qp{j}")
            nc.gpsimd.memset(qp[:], 0.0)
            nc.gpsimd.tensor_scalar_mul(qp[0:64, 0:64], qsum[0:64, :], SCALE / 9.0)
            nc.gpsimd.tensor_scalar_mul(
                qp[64:128, 64:128], qsum[64:128, :], SCALE / 9.0)
            S.QpBD[j] = qp

        def qk_chunk(S, cc):
            qt = sb.tile([128, 640], BF16, tag=f"qkt{cc}", bufs=2,
                         name=f"qkt{cc}")
            ccs = slice(128 * cc, 128 * (cc + 1))
            for ci, (n0, nw) in enumerate(NF):
                ps = ps_big.tile([128, 512], F32, tag="pbig")
                k = 0
                # q/k channels use the cheapest 1-term fp8 product: the
                # quantization error only perturbs softmax logits a few
                # percent, which the attention normalization damps to well
                # under the error budget. V keeps all 3 terms.
                for pc in range(4):
                    nc.tensor.matmul(
                        ps[:, 0:nw],
                        WQH[:, pc, :, ccs],
                        S.XH[:, pc, :, n0 : n0 + nw],
                        start=(pc == 0),
                        stop=(pc == 3),
                        perf_mode=DR,
                    )
                if (cc + ci) % 2 == 0:
                    nc.vector.tensor_copy(qt[:, n0 : n0 + nw], ps[:, 0:nw])
                else:
                    nc.scalar.copy(qt[:, n0 : n0 + nw], ps[:, 0:nw])
            nc.gpsimd.memset(qt[:, 577:640], 0.0)
            S.qkT[cc] = qt
            if cc < 8:
                pool_chunk(S, cc)

        def v_chunk(S, t):
            toff, rows = TOK[t]
            vt = sb.tile([128, 8, 129], BF16, tag=f"v{t}", bufs=2, name=f"v{t}")
            lo = toff if rows == 128 else 512
            for h2 in range(2):
                ps = ps_big.tile([128, 512], F32, tag="pbig")
                k = 0
                for (wt, off, xt) in (
                    (WQH, 2048, S.XH), (WQH, 2048, S.XL), (WQL, 0, S.X16)
                ):
                    for pc in range(4):
                        nc.tensor.matmul(
                            ps[:],
                            xt[:, pc, :, lo : lo + 128],
                            wt[:, pc, :, off + 512 * h2 : off + 512 * (h2 + 1)],
                            start=(k == 0),
                            stop=(k == 11),
                            perf_mode=DR,
                        )
                        k += 1
                nc.any.tensor_copy(
                    vt[:, 4 * h2 : 4 * (h2 + 1), 0:128],
                    ps[:].rearrange("p (a b) -> p a b", a=4),
                )
            # 1/16 makes the s1 denominator den/16, so Qd evicts as 16*Qd:
            # the out_mm psum then sits in a healthy fp8 range for the
            # hi/lo split (host divides the output by 16).
            nc.gpsimd.memset(vt[:, :, 128:129], 1.0 / 16.0)
            S.V[t] = vt

        def qkv_work(S):
            w = [(lambda S=S, cc=cc: qk_chunk(S, cc)) for cc in range(16)]
            w += [(lambda S=S, t=t: v_chunk(S, t)) for t in range(5)]
            return w

        def s1_score(S, pp):
            e1t = sb.tile([128, 5, 128], BF16, tag="e1t", bufs=2)
            # zero the tail-chunk pad rows; the exp below rewrites row 64
            nc.gpsimd.memset(e1t[64:128, 4, :], 0.0)
            psa = ps_big.tile([128, 512], F32, tag="pbig")
            for t in range(4):
                nc.tensor.matmul(
                    psa[:, 128 * t : 128 * (t + 1)],
                    S.qkT[8 + pp][:, 128 * t : 128 * (t + 1)],
                    S.QpBD[pp][:],
                    start=True,
                    stop=True,
                )
            psb = ps_small.tile([128, 132], F32, tag="psmall")
            nc.tensor.matmul(
                psb[0:65, 0:128],
                S.qkT[8 + pp][:, 512:577],
                S.QpBD[pp][:],
                start=True,
                stop=True,
            )
            nc.scalar.activation(
                e1t[:, 0:4, :].rearrange("p a b -> p (a b)"), psa[:], EXP
            )
            nc.scalar.activation(e1t[0:65, 4, :], psb[0:65, 0:128], EXP)
            S.s1e[pp] = e1t

        def s1_qd(S, pp):
            e1t = S.s1e.pop(pp)
            psq = ps_small.tile([128, 132], F32, tag="psmall")
            for t in range(5):
                nc.tensor.matmul(
                    psq[:, 0:129],
                    e1t[:, t, :],
                    S.V[t][:, pp, :],
                    start=(t == 0),
                    stop=(t == 4),
                )
            r1 = sb.tile([128, 1], F32, tag="r1", bufs=4)
            nc.vector.reciprocal(r1[:], psq[:, 128:129])
            qd = sb.tile([128, 128], BF16, tag=f"qd{pp}", bufs=2, name=f"qd{pp}")
            nc.gpsimd.memset(qd[:], 0.0)
            nc.vector.tensor_scalar_mul(
                qd[0:64, 0:64], psq[0:64, 0:64], r1[0:64, 0:1]
            )
            nc.vector.tensor_scalar_mul(
                qd[64:128, 64:128], psq[64:128, 64:128], r1[64:128, 0:1]
            )
            S.QdBD[pp] = qd

        def s2_tok(S, oc, t):
            toff, rows = TOK[t]
            lo = toff if rows == 128 else 512
            ps = ps_big.tile([128, 512], F32, tag="pbig")
            for pz in range(4):
                pp = 4 * oc + pz
                nc.tensor.matmul(
                    ps[:, 128 * pz : 128 * (pz + 1)],
                    S.qkT[pp][:, lo : lo + 128],
                    S.QpBD[pp][:],
                    start=True,
                    stop=True,
                )
            s2e = sb.tile([128, 512], BF16, tag="s2e", bufs=4)
            nc.scalar.activation(s2e[0:rows, :], ps[0:rows, :], EXP)
            s2s = sb.tile([128, 8], F32, tag="s2s", bufs=4)
            nc.vector.reduce_sum(
                s2s[0:rows, :],
                s2e[0:rows, :].rearrange("p (h q) -> p h q", q=64),
                axis=mybir.AxisListType.X,
            )
            r2 = sb.tile([128, 8], F32, tag="r2", bufs=4)
            nc.vector.reciprocal(r2[0:rows, :], s2s[0:rows, :])
            a2 = sb.tile([128, 512], BF16, tag=f"a2n{t}_{oc}", bufs=1,
                         name=f"a2n{t}_{oc}")
            if rows < 128:
                # pad rows must be zero: the transpose DMA moves all 128 rows
                nc.gpsimd.memset(a2[64:128, :], 0.0)
            for hz in range(2):
                eng = nc.vector if hz == 0 else nc.gpsimd
                zs = slice(256 * hz, 256 * (hz + 1))
                eng.tensor_tensor(
                    a2[0:rows, zs].rearrange("p (h q) -> p h q", q=64),
                    s2e[0:rows, zs].rearrange("p (h q) -> p h q", q=64),
                    r2[0:rows, 4 * hz : 4 * hz + 4]
                    .unsqueeze(2)
                    .broadcast_to((rows, 4, 64)),
                    op=mybir.AluOpType.mult,
                )
            S.A2n[t][oc] = a2

        def a2_transpose(S, pp):
            oc, sl = pp // 4, 128 * (pp % 4)
            pa = ps_big.tile([128, 512], BF16, tag="pbig")
            for t in range(4):
                nc.tensor.transpose(
                    pa[:, 128 * t : 128 * (t + 1)],
                    S.A2n[t][oc][:, sl : sl + 128],
                    ident[:],
                )
            pb = ps_small.tile([128, 132], BF16, tag="psmall")
            # tail pad rows of A2n are zeroed, so the full block transposes
            # cleanly and a2t needs no pad memset
            nc.tensor.transpose(
                pb[:, 0:128], S.A2n[4][oc][:, sl : sl + 128], ident[:]
            )
            a2t = sb.tile([128, 640], BF16, tag="a2t", bufs=3)
            nc.any.tensor_copy(a2t[:, 0:512], pa[:])
            nc.any.tensor_copy(a2t[:, 512:640], pb[:, 0:128])
            S.a2t[pp] = a2t

        def out_mm(S, pp):
            a2t = S.a2t.pop(pp)
            pq, i = pp // 2, pp % 2
            oa = ps_big.tile([128, 512], F32, tag="pbig")
            ob = ps_small.tile([128, 132], F32, tag="psmall")
            nc.tensor.matmul(
                oa[:], S.QdBD[pp][:], a2t[:, 0:512], start=True, stop=True
            )
            nc.tensor.matmul(
                ob[:, 0:128], S.QdBD[pp][:], a2t[:, 512:640], start=True,
                stop=True,
            )
            nc.scalar.copy(S.OTH[pq][:, i, 0:512], oa[:])
            nc.scalar.copy(S.OTH[pq][:, i, 512:640], ob[:, 0:128])
            nc.vector.tensor_tensor(
                S.OTL[pq][:, i, 0:512], oa[:], S.OTH[pq][:, i, 0:512], op=SUB
            )
            nc.vector.tensor_tensor(
                S.OTL[pq][:, i, 512:640], ob[:, 0:128],
                S.OTH[pq][:, i, 512:640], op=SUB,
            )
            nc.gpsimd.tensor_scalar_mul(
                S.OT16[pq][:, i, :], S.OTH[pq][:, i, :], 1.0 / 16.0
            )

        def proj_chunk(S, t):
            toff, rows = TOK[t]
            lo = toff if rows == 128 else 512
            y = sb.tile([128, 1024], BF16, tag="y", bufs=2)
            for half in range(2):
                cs = slice(512 * half, 512 * (half + 1))
                ps = ps_big.tile([128, 512], F32, tag="pbig")
                k = 0
                for pq in range(4):
                    for (ot, wt) in ((S.OTH, WPH), (S.OTL, WPH), (S.OT16, WPL)):
                        nc.tensor.matmul(
                            ps[:],
                            ot[pq][:, :, lo : lo + 128],
                            wt[:, pq, :, cs],
                            start=(k == 0),
                            stop=False,
                            perf_mode=DR,
                        )
                        k += 1
                # bias (x16) joins the accumulation via a K=1 matmul so the
                # evict is a plain copy, splittable across DVE and Act
                nc.tensor.matmul(ps[:], ones[0:1, :], wb[0:1, cs],
                                 start=False, stop=True)
                if (2 * t + half) % 2 == 0:
                    nc.vector.tensor_copy(y[0:rows, cs], ps[0:rows, :])
                else:
                    nc.scalar.copy(y[0:rows, cs], ps[0:rows, :])
            nc.sync.dma_start(out_d[S.b, toff : toff + rows, :], y[0:rows, :])

        # ------------------------------------------------------------------
        # software-pipelined emission: batch b's attention middle is
        # interleaved with batch b+1's QKV/V chunk groups as PE filler.
        # ------------------------------------------------------------------
        S = make_state(order[0], X0)
        for w in qkv_work(S):
            w()

        for bi, b in enumerate(order):
            filler = []
            if bi + 1 < len(order):
                xn = pre_x.pop(bi + 1, None) or dma_x(order[bi + 1])
                Sn = make_state(order[bi + 1], xn)
                filler = qkv_work(Sn)
            fi = [0]

            def step(n=1):
                for _ in range(n):
                    if fi[0] < len(filler):
                        filler[fi[0]]()
                        fi[0] += 1

            # stage 1, with stage-2 chunks spread across the pair loop and
            # transpose-DMAs issued as soon as each octet's A2n completes
            for pp in range(8):
                s1_score(S, pp)
                if pp > 0:
                    s1_qd(S, pp - 1)
                if 1 <= pp <= 5:
                    s2_tok(S, 0, pp - 1)
                if 2 <= pp <= 6:
                    s2_tok(S, 1, pp - 2)
                step()
            s1_qd(S, 7)

            # transposes + outT matmuls (2-pair lag) + fp8 split evicts
            a2_transpose(S, 0)
            a2_transpose(S, 1)
            step()
            for pp in range(2, 8):
                a2_transpose(S, pp)
                out_mm(S, pp - 2)
                step()
            out_mm(S, 6)
            out_mm(S, 7)
            step()

            # output projection
            for t in range(5):
                proj_chunk(S, t)
                step()
            step(len(filler))

            if filler:
                S = Sn

        for p in (ps_small, ps_big, sb, w_pool, const_pool):
            p.release()

    nc.compile()
    return nc


_NC_CACHE = {}


def _get_nc(nb: int = NB):
    if nb not in _NC_CACHE:
        _NC_CACHE[nb] = build_program(nb)
    return _NC_CACHE[nb]


def _ilv_k(a):
    """[K, F] -> [128, K//256, 2, F], partition-first with K-chunk pairs
    (2j, 2j+1) in the two DoubleRow planes."""
    Kd, F = a.shape
    return np.ascontiguousarray(a.reshape(Kd // 256, 2, 128, F).transpose(2, 0, 1, 3))


def kernel(X, W_qkv, W_proj, b_proj, layer_idx=None):
    assert X.shape == (B, N, C)
    nc = _get_nc(NB)
    xt = np.zeros((B, C, 640), dtype=np.float32)
    xt[:, :, :N] = np.asarray(X, dtype=np.float32).transpose(0, 2, 1)
    xhi = xt.astype(NP8)
    xhf = xhi.astype(np.float32)
    xlo = (xt - xhf).astype(NP8)
    xh16 = (xhf / 16.0).astype(NP8)

    def ilv_x(a):
        # [B, 1024, 640] -> [B, 128, 4, 2, 640]
        return np.ascontiguousarray(
            a.reshape(B, 4, 2, 128, 640).transpose(0, 3, 1, 2, 4)
        )

    wq = np.ascontiguousarray(np.asarray(W_qkv, dtype=np.float32).T)
    wqh = wq.astype(NP8)
    wql16 = (16.0 * (wq - wqh.astype(np.float32))).astype(NP8)
    wp = np.ascontiguousarray(np.asarray(W_proj, dtype=np.float32).T)
    wph = wp.astype(NP8)
    wpl16 = (16.0 * (wp - wph.astype(np.float32))).astype(NP8)
    wbias = (16.0 * np.asarray(b_proj, dtype=np.float32)).reshape(1, C).astype(
        ml_dtypes.bfloat16
    )
    xhi_i, xlo_i, xh16_i = ilv_x(xhi), ilv_x(xlo), ilv_x(xh16)
    in_maps = [
        {
            "xhi": xhi_i[NB * i : NB * (i + 1)],
            "xlo": xlo_i[NB * i : NB * (i + 1)],
            "xh16": xh16_i[NB * i : NB * (i + 1)],
            "wqh": _ilv_k(wqh),
            "wql": _ilv_k(wql16),
            "wph": _ilv_k(wph),
            "wpl": _ilv_k(wpl16),
            "wbias": wbias,
        }
        for i in range(N_CORES)
    ]
    res = run_bass_kernel_spmd(nc, in_maps, core_ids=list(range(N_CORES)))
    out = np.concatenate(
        [res.results[i]["out"].astype(np.float32) for i in range(N_CORES)], axis=0
    )
    return (out / 16.0).astype(np.float32)


if __name__ == "__main__":
    rng = np.random.default_rng(0)
    X = rng.standard_normal((B, N, C), dtype=np.float32)
    W_qkv = rng.standard_normal((3 * C, C), dtype=np.float32) * C**-0.5
    W_proj = rng.standard_normal((C, C), dtype=np.float32) * C**-0.5
    b_proj = np.zeros(C, dtype=np.float32)
    out = kernel(X, W_qkv, W_proj, b_proj, 1)
    print(out.shape, out.dtype)


# revision 37
# speedup vs baseline: 1.0335x; 1.0335x over previous
"""Trainium2 Bass kernel for two-stage pooled-query attention.

Problem (hardcoded):
    B=32, N=577, C=1024, H=16 heads, d=64, pooled queries 8x8 (3x3 mean over
    24x24 grid of non-cls tokens).
    qkv = X @ W_qkv.T ; pool Xq -> Qp ; s1 = softmax(Qp*s @ K^T) @ V ;
    s2 = softmax(Xq*s @ Qp^T) @ s1 ; out = s2 @ W_proj.T + b_proj

Strategy: pure data-parallel over batch across 8 NeuronCores (4 batches per
core, no collectives).

The two big GEMMs (QKV and output projection, ~75% of matmul cycles) run as
3-term fp8e4 DoubleRow matmuls: A@B ~ Ahi@Bhi + Alo@Bhi + (Ahi/16)@(16*Blo),
with K-chunk pairs packed into each DoubleRow instruction (effective K=256
per instruction at 0.5 cycles/row -> 4x bf16 throughput per K=128 product,
so the 3-term scheme runs at 0.75x bf16 cost with ~bf16 accuracy). X and the
weights arrive pre-split/interleaved from the host (layout prep); the
attention output is split on-chip during the PSUM evict, pre-scaled x16 (via
a 1/16 ones-column in V that scales Qd) so the residual stays out of fp8's
subnormal-flush range; the host divides the output by 16.

Attention middle stays bf16 (contractions are K<=128 there, DoubleRow does
not apply). Stage 1 is computed transposed relative to the baseline:
scores E1T[n, 2q] = exp(K^T Qp) directly (no PE transpose of A1), and the
softmax denominator comes from the ones-column in V, so Qd = E1T^T @ [V | 1]
yields numerator and denominator in one accumulation.

The whole thing is software-pipelined across batches: batch b+1's QKV/V
chunk-groups are emitted as PE filler interleaved into batch b's attention
middle (qkT/V/X are double-buffered), so the PE never drains during the
dependency-heavy softmax stretches.
"""

import os
import sys

import numpy as np

sys.path.insert(0, "/opt/trn_rl_repo")

import ml_dtypes  # noqa: E402

import concourse.tile as tile  # noqa: E402
from concourse import bacc, mybir  # noqa: E402
from concourse.bass_utils import run_bass_kernel_spmd  # noqa: E402
from concourse.masks import make_identity  # noqa: E402

B, N, C = 32, 577, 1024
H, D = 16, 64
SCALE = D ** -0.5
N_CORES = 8
NB = B // N_CORES  # batches per core

BF16 = mybir.dt.bfloat16
FP8 = mybir.dt.float8e4
F32 = mybir.dt.float32
DR = mybir.MatmulPerfMode.DoubleRow
NP8 = ml_dtypes.float8_e4m3

# token chunks of 577 = 4*128 + 65
TOK = [(0, 128), (128, 128), (256, 128), (384, 128), (512, 65)]
# free-dim chunks of 577 for wide matmuls / psum banks
NF = [(0, 320), (320, 257)]
EXP = mybir.ActivationFunctionType.Exp
SUB = mybir.AluOpType.subtract


def build_program(nb: int = NB):
    nc = bacc.Bacc("TRN2", target_bir_lowering=False, debug=False)

    # X pre-transposed [c, n], zero-padded to 640 tokens, split hi/lo/hi16
    # fp8 and K-pair interleaved: [pc, part, plane, tok].
    xhi_d = nc.dram_tensor("xhi", [nb, 128, 4, 2, 640], FP8, kind="ExternalInput")
    xlo_d = nc.dram_tensor("xlo", [nb, 128, 4, 2, 640], FP8, kind="ExternalInput")
    xh16_d = nc.dram_tensor("xh16", [nb, 128, 4, 2, 640], FP8, kind="ExternalInput")
    wqh_d = nc.dram_tensor("wqh", [128, 4, 2, 3 * C], FP8, kind="ExternalInput")
    wql_d = nc.dram_tensor("wql", [128, 4, 2, 3 * C], FP8, kind="ExternalInput")
    wph_d = nc.dram_tensor("wph", [128, 4, 2, C], FP8, kind="ExternalInput")
    wpl_d = nc.dram_tensor("wpl", [128, 4, 2, C], FP8, kind="ExternalInput")
    wbias_d = nc.dram_tensor("wbias", [1, C], BF16, kind="ExternalInput")
    out_d = nc.dram_tensor("out", [nb, N, C], BF16, kind="ExternalOutput")

    with tile.TileContext(nc) as tc:
        const_pool = tc.alloc_tile_pool(name="const", bufs=1)
        w_pool = tc.alloc_tile_pool(name="w", bufs=1)
        sb = tc.alloc_tile_pool(name="sb", bufs=2)
        ps_big = tc.alloc_tile_pool(name="ps_big", bufs=5, space="PSUM")
        ps_small = tc.alloc_tile_pool(name="ps_small", bufs=3, space="PSUM")

        ident = const_pool.tile([128, 128], BF16, tag="ident")
        make_identity(nc, ident[:])
        ones = const_pool.tile([1, 128], BF16, tag="ones")
        nc.gpsimd.memset(ones[:], 1.0)

        def dma_x_grp(b, grp):
            """DMA one of the three X tensors (0=hi, 1=lo, 2=hi16)."""
            tag, dram = [("xh", xhi_d), ("xl", xlo_d), ("x16", xh16_d)][grp]
            t = sb.tile([128, 4, 2, 640], FP8, tag=tag, bufs=2, name=tag)
            nc.sync.dma_start(t[:], dram[b])
            return t

        def dma_x(b):
            return tuple(dma_x_grp(b, g) for g in range(3))

        # weights stream in 512-col chunks interleaved with batch-0's X in
        # the order the QKV gemm consumes them (term-major: hi*hi first).
        WQH = w_pool.tile([128, 4, 2, 3 * C], FP8, tag="wqh", name="wqh")
        # WQL holds only the V columns: the q/k channels use the 2-term
        # product, so the lo-correction weights for them are never read
        WQL = w_pool.tile([128, 4, 2, C], FP8, tag="wql", name="wql")

        def dma_wq(which, blk):
            cs = slice(512 * blk, 512 * (blk + 1))
            if which == 0:
                nc.sync.dma_start(WQH[:, :, :, cs], wqh_d[:, :, :, cs])
            else:
                nc.sync.dma_start(
                    WQL[:, :, :, cs],
                    wql_d[:, :, :, 2048 + 512 * blk : 2048 + 512 * (blk + 1)],
                )

        repeat = int(os.environ.get("KERNEL_REPEAT", "1"))
        order = [bb for _ in range(repeat) for bb in range(nb)]

        def dma_x_grp_split(b, grp):
            tag, dram = [("xh", xhi_d), ("xl", xlo_d), ("x16", xh16_d)][grp]
            t = sb.tile([128, 4, 2, 640], FP8, tag=tag, bufs=2, name=tag)
            nc.sync.dma_start(t[:, 0:2], dram[b, :, 0:2])
            return t

        def finish_x_grp(t, b, grp):
            dram = [xhi_d, xlo_d, xh16_d][grp]
            nc.sync.dma_start(t[:, 2:4], dram[b, :, 2:4])

        XH0 = dma_x_grp_split(order[0], 0)
        nc.sync.dma_start(WQH[:, 0:2, :, 0:512], wqh_d[:, 0:2, :, 0:512])
        finish_x_grp(XH0, order[0], 0)
        nc.sync.dma_start(WQH[:, 2:4, :, 0:512], wqh_d[:, 2:4, :, 0:512])
        XL0 = dma_x_grp(order[0], 1)
        for blk in range(1, 4):
            dma_wq(0, blk)
        X160 = dma_x_grp(order[0], 2)
        X0 = (XH0, XL0, X160)
        dma_wq(0, 4)
        dma_wq(0, 5)
        dma_wq(1, 0)
        dma_wq(1, 1)
        # batch 1's X goes out before the proj weights: the pipelined filler
        # needs it early, the proj weights only by the first proj phase
        pre_x = {}
        if len(order) > 1:
            pre_x[1] = dma_x(order[1])
        WPH = w_pool.tile([128, 4, 2, C], FP8, tag="wph", name="wph")
        WPL = w_pool.tile([128, 4, 2, C], FP8, tag="wpl", name="wpl")
        nc.sync.dma_start(WPH[:], wph_d[:])
        nc.sync.dma_start(WPL[:], wpl_d[:])
        wb = w_pool.tile([1, C], BF16, tag="wb")
        nc.sync.dma_start(wb[:], wbias_d[:])

        # bias broadcast [128, 1024]; built lazily (first use is phase 8)
        bias = const_pool.tile([128, C], BF16, tag="bias")
        bias_built = [False]

        def build_bias():
            if bias_built[0]:
                return
            bias_built[0] = True
            for half in range(2):
                cs = slice(512 * half, 512 * (half + 1))
                bps = ps_big.tile([128, 512], F32, tag="pbig")
                nc.tensor.matmul(
                    bps[:], ones[0:1, :], wb[0:1, cs], start=True, stop=True
                )
                nc.any.tensor_copy(bias[:, cs], bps[:])

        # ------------------------------------------------------------------
        # per-batch state + emitters
        # ------------------------------------------------------------------
        class St:
            pass

        def make_state(b, X3):
            S = St()
            S.b = b
            S.XH, S.XL, S.X16 = X3
            S.qkT = [None] * 16
            S.QpBD = [None] * 8
            S.V = [None] * 5
            S.QdBD = [None] * 8
            S.A2n = [[None, None] for _ in range(5)]
            S.s1e = {}
            S.a2t = {}
            S.OTH = [sb.tile([128, 2, 640], FP8, tag=f"oth{pq}", bufs=1,
                             name=f"oth{pq}") for pq in range(4)]
            S.OTL = [sb.tile([128, 2, 640], FP8, tag=f"otl{pq}", bufs=1,
                             name=f"otl{pq}") for pq in range(4)]
            S.OT16 = [sb.tile([128, 2, 640], FP8, tag=f"ot16{pq}", bufs=1,
                              name=f"ot16{pq}") for pq in range(4)]
            return S

        def pool_chunk(S, j):
            qsum = sb.tile([128, 64], F32, tag="qsum", bufs=3)
            view = S.qkT[j][:, 0:576].rearrange(
                "p (pr dr pc dc) -> p pr pc dr dc", pr=8, dr=3, pc=8, dc=3
            )
            nc.vector.reduce_sum(qsum[:], view, axis=mybir.AxisListType.XY)
            qp = sb.tile([128, 128], BF16, tag=f"qp{j}", bufs=2, name=f"qp{j}")
            nc.gpsimd.memset(qp[:], 0.0)
            nc.gpsimd.tensor_scalar_mul(qp[0:64, 0:64], qsum[0:64, :], SCALE / 9.0)
            nc.gpsimd.tensor_scalar_mul(
                qp[64:128, 64:128], qsum[64:128, :], SCALE / 9.0)
            S.QpBD[j] = qp

        def qk_chunk(S, cc):
            qt = sb.tile([128, 640], BF16, tag=f"qkt{cc}", bufs=2,
                         name=f"qkt{cc}")
            ccs = slice(128 * cc, 128 * (cc + 1))
            for ci, (n0, nw) in enumerate(NF):
                ps = ps_big.tile([128, 512], F32, tag="pbig")
                k = 0
                # q/k channels use the cheapest 1-term fp8 product: the
                # quantization error only perturbs softmax logits a few
                # percent, which the attention normalization damps to well
                # under the error budget. V keeps all 3 terms.
                for pc in range(4):
                    nc.tensor.matmul(
                        ps[:, 0:nw],
                        WQH[:, pc, :, ccs],
                        S.XH[:, pc, :, n0 : n0 + nw],
                        start=(pc == 0),
                        stop=(pc == 3),
                        perf_mode=DR,
                    )
                if (cc + ci) % 2 == 0:
                    nc.vector.tensor_copy(qt[:, n0 : n0 + nw], ps[:, 0:nw])
                else:
                    nc.scalar.copy(qt[:, n0 : n0 + nw], ps[:, 0:nw])
            nc.gpsimd.memset(qt[:, 577:640], 0.0)
            S.qkT[cc] = qt
            if cc < 8:
                pool_chunk(S, cc)

        def v_chunk(S, t):
            toff, rows = TOK[t]
            vt = sb.tile([128, 8, 129], BF16, tag=f"v{t}", bufs=2, name=f"v{t}")
            lo = toff if rows == 128 else 512
            for h2 in range(2):
                ps = ps_big.tile([128, 512], F32, tag="pbig")
                k = 0
                for (wt, off, xt) in (
                    (WQH, 2048, S.XH), (WQH, 2048, S.XL), (WQL, 0, S.X16)
                ):
                    for pc in range(4):
                        nc.tensor.matmul(
                            ps[:],
                            xt[:, pc, :, lo : lo + 128],
                            wt[:, pc, :, off + 512 * h2 : off + 512 * (h2 + 1)],
                            start=(k == 0),
                            stop=(k == 11),
                            perf_mode=DR,
                        )
                        k += 1
                nc.any.tensor_copy(
                    vt[:, 4 * h2 : 4 * (h2 + 1), 0:128],
                    ps[:].rearrange("p (a b) -> p a b", a=4),
                )
            # 1/16 makes the s1 denominator den/16, so Qd evicts as 16*Qd:
            # the out_mm psum then sits in a healthy fp8 range for the
            # hi/lo split (host divides the output by 16).
            nc.gpsimd.memset(vt[:, :, 128:129], 1.0 / 16.0)
            S.V[t] = vt

        def qkv_work(S):
            w = [(lambda S=S, cc=cc: qk_chunk(S, cc)) for cc in range(16)]
            w += [(lambda S=S, t=t: v_chunk(S, t)) for t in range(5)]
            return w

        def s1_score(S, pp):
            e1t = sb.tile([128, 5, 128], BF16, tag="e1t", bufs=2)
            # zero the tail-chunk pad rows; the exp below rewrites row 64
            nc.gpsimd.memset(e1t[64:128, 4, :], 0.0)
            psa = ps_big.tile([128, 512], F32, tag="pbig")
            for t in range(4):
                nc.tensor.matmul(
                    psa[:, 128 * t : 128 * (t + 1)],
                    S.qkT[8 + pp][:, 128 * t : 128 * (t + 1)],
                    S.QpBD[pp][:],
                    start=True,
                    stop=True,
                )
            psb = ps_small.tile([128, 132], F32, tag="psmall")
            nc.tensor.matmul(
                psb[0:65, 0:128],
                S.qkT[8 + pp][:, 512:577],
                S.QpBD[pp][:],
                start=True,
                stop=True,
            )
            nc.scalar.activation(
                e1t[:, 0:4, :].rearrange("p a b -> p (a b)"), psa[:], EXP
            )
            nc.scalar.activation(e1t[0:65, 4, :], psb[0:65, 0:128], EXP)
            S.s1e[pp] = e1t

        def s1_qd(S, pp):
            e1t = S.s1e.pop(pp)
            psq = ps_small.tile([128, 132], F32, tag="psmall")
            for t in range(5):
                nc.tensor.matmul(
                    psq[:, 0:129],
                    e1t[:, t, :],
                    S.V[t][:, pp, :],
                    start=(t == 0),
                    stop=(t == 4),
                )
            r1 = sb.tile([128, 1], F32, tag="r1", bufs=4)
            nc.vector.reciprocal(r1[:], psq[:, 128:129])
            qd = sb.tile([128, 128], BF16, tag=f"qd{pp}", bufs=2, name=f"qd{pp}")
            nc.gpsimd.memset(qd[:], 0.0)
            nc.vector.tensor_scalar_mul(
                qd[0:64, 0:64], psq[0:64, 0:64], r1[0:64, 0:1]
            )
            nc.vector.tensor_scalar_mul(
                qd[64:128, 64:128], psq[64:128, 64:128], r1[64:128, 0:1]
            )
            S.QdBD[pp] = qd

        def s2_tok(S, oc, t):
            toff, rows = TOK[t]
            lo = toff if rows == 128 else 512
            ps = ps_big.tile([128, 512], F32, tag="pbig")
            for pz in range(4):
                pp = 4 * oc + pz
                nc.tensor.matmul(
                    ps[:, 128 * pz : 128 * (pz + 1)],
                    S.qkT[pp][:, lo : lo + 128],
                    S.QpBD[pp][:],
                    start=True,
                    stop=True,
                )
            s2e = sb.tile([128, 512], BF16, tag="s2e", bufs=4)
            nc.scalar.activation(s2e[0:rows, :], ps[0:rows, :], EXP)
            s2s = sb.tile([128, 8], F32, tag="s2s", bufs=4)
            nc.vector.reduce_sum(
                s2s[0:rows, :],
                s2e[0:rows, :].rearrange("p (h q) -> p h q", q=64),
                axis=mybir.AxisListType.X,
            )
            r2 = sb.tile([128, 8], F32, tag="r2", bufs=4)
            nc.vector.reciprocal(r2[0:rows, :], s2s[0:rows, :])
            a2 = sb.tile([128, 512], BF16, tag=f"a2n{t}_{oc}", bufs=1,
                         name=f"a2n{t}_{oc}")
            if rows < 128:
                # pad rows must be zero: the transpose DMA moves all 128 rows
                nc.gpsimd.memset(a2[64:128, :], 0.0)
            for hz in range(2):
                eng = nc.vector if hz == 0 else nc.gpsimd
                zs = slice(256 * hz, 256 * (hz + 1))
                eng.tensor_tensor(
                    a2[0:rows, zs].rearrange("p (h q) -> p h q", q=64),
                    s2e[0:rows, zs].rearrange("p (h q) -> p h q", q=64),
                    r2[0:rows, 4 * hz : 4 * hz + 4]
                    .unsqueeze(2)
                    .broadcast_to((rows, 4, 64)),
                    op=mybir.AluOpType.mult,
                )
            S.A2n[t][oc] = a2

        def a2_transpose(S, pp):
            oc, sl = pp // 4, 128 * (pp % 4)
            pa = ps_big.tile([128, 512], BF16, tag="pbig")
            for t in range(4):
                nc.tensor.transpose(
                    pa[:, 128 * t : 128 * (t + 1)],
                    S.A2n[t][oc][:, sl : sl + 128],
                    ident[:],
                )
            pb = ps_small.tile([128, 132], BF16, tag="psmall")
            # tail pad rows of A2n are zeroed, so the full block transposes
            # cleanly and a2t needs no pad memset
            nc.tensor.transpose(
                pb[:, 0:128], S.A2n[4][oc][:, sl : sl + 128], ident[:]
            )
            a2t = sb.tile([128, 640], BF16, tag="a2t", bufs=3)
            nc.any.tensor_copy(a2t[:, 0:512], pa[:])
            nc.any.tensor_copy(a2t[:, 512:640], pb[:, 0:128])
            S.a2t[pp] = a2t

        def out_mm(S, pp):
            a2t = S.a2t.pop(pp)
            pq, i = pp // 2, pp % 2
            oa = ps_big.tile([128, 512], F32, tag="pbig")
            ob = ps_small.tile([128, 132], F32, tag="psmall")
            nc.tensor.matmul(
                oa[:], S.QdBD[pp][:], a2t[:, 0:512], start=True, stop=True
            )
            nc.tensor.matmul(
                ob[:, 0:128], S.QdBD[pp][:], a2t[:, 512:640], start=True,
                stop=True,
            )
            nc.scalar.copy(S.OTH[pq][:, i, 0:512], oa[:])
            nc.scalar.copy(S.OTH[pq][:, i, 512:640], ob[:, 0:128])
            nc.vector.tensor_tensor(
                S.OTL[pq][:, i, 0:512], oa[:], S.OTH[pq][:, i, 0:512], op=SUB
            )
            nc.vector.tensor_tensor(
                S.OTL[pq][:, i, 512:640], ob[:, 0:128],
                S.OTH[pq][:, i, 512:640], op=SUB,
            )
            nc.gpsimd.tensor_scalar_mul(
                S.OT16[pq][:, i, :], S.OTH[pq][:, i, :], 1.0 / 16.0
            )

        def proj_chunk(S, t):
            toff, rows = TOK[t]
            lo = toff if rows == 128 else 512
            y = sb.tile([128, 1024], BF16, tag="y", bufs=2)
            for half in range(2):
                cs = slice(512 * half, 512 * (half + 1))
                ps = ps_big.tile([128, 512], F32, tag="pbig")
                k = 0
                for pq in range(4):
                    for (ot, wt) in ((S.OTH, WPH), (S.OTL, WPH), (S.OT16, WPL)):
                        nc.tensor.matmul(
                            ps[:],
                            ot[pq][:, :, lo : lo + 128],
                            wt[:, pq, :, cs],
                            start=(k == 0),
                            stop=(k == 11),
                            perf_mode=DR,
                        )
                        k += 1
                nc.vector.tensor_add(y[0:rows, cs], ps[0:rows, :],
                                     bias[0:rows, cs])
            nc.sync.dma_start(out_d[S.b, toff : toff + rows, :], y[0:rows, :])

        # ------------------------------------------------------------------
        # software-pipelined emission: batch b's attention middle is
        # interleaved with batch b+1's QKV/V chunk groups as PE filler.
        # ------------------------------------------------------------------
        S = make_state(order[0], X0)
        for w in qkv_work(S):
            w()

        for bi, b in enumerate(order):
            filler = []
            if bi + 1 < len(order):
                xn = pre_x.pop(bi + 1, None) or dma_x(order[bi + 1])
                Sn = make_state(order[bi + 1], xn)
                filler = qkv_work(Sn)
            fi = [0]

            def step(n=1):
                for _ in range(n):
                    if fi[0] < len(filler):
                        filler[fi[0]]()
                        fi[0] += 1

            # stage 1, with stage-2 chunks spread across the pair loop and
            # transpose-DMAs issued as soon as each octet's A2n completes
            for pp in range(8):
                s1_score(S, pp)
                if pp > 0:
                    s1_qd(S, pp - 1)
                if 1 <= pp <= 5:
                    s2_tok(S, 0, pp - 1)
                if 2 <= pp <= 6:
                    s2_tok(S, 1, pp - 2)
                step()
            s1_qd(S, 7)

            # transposes + outT matmuls (2-pair lag) + fp8 split evicts
            a2_transpose(S, 0)
            a2_transpose(S, 1)
            step()
            for pp in range(2, 8):
                a2_transpose(S, pp)
                out_mm(S, pp - 2)
                step()
            out_mm(S, 6)
            out_mm(S, 7)
            step()

            # output projection
            build_bias()
            for t in range(5):
                proj_chunk(S, t)
                step()
            step(len(filler))

            if filler:
                S = Sn

        for p in (ps_small, ps_big, sb, w_pool, const_pool):
            p.release()

    nc.compile()
    return nc


_NC_CACHE = {}


def _get_nc(nb: int = NB):
    if nb not in _NC_CACHE:
        _NC_CACHE[nb] = build_program(nb)
    return _NC_CACHE[nb]


def _ilv_k(a):
    """[K, F] -> [128, K//256, 2, F], partition-first with K-chunk pairs
    (2j, 2j+1) in the two DoubleRow planes."""
    Kd, F = a.shape
    return np.ascontiguousarray(a.reshape(Kd // 256, 2, 128, F).transpose(2, 0, 1, 3))


def kernel(X, W_qkv, W_proj, b_proj, layer_idx=None):
    assert X.shape == (B, N, C)
    nc = _get_nc(NB)
    xt = np.zeros((B, C, 640), dtype=np.float32)
    xt[:, :, :N] = np.asarray(X, dtype=np.float32).transpose(0, 2, 1)
    xhi = xt.astype(NP8)
    xhf = xhi.astype(np.float32)
    xlo = (xt - xhf).astype(NP8)
    xh16 = (xhf / 16.0).astype(NP8)

    def ilv_x(a):
        # [B, 1024, 640] -> [B, 128, 4, 2, 640]
        return np.ascontiguousarray(
            a.reshape(B, 4, 2, 128, 640).transpose(0, 3, 1, 2, 4)
        )

    wq = np.ascontiguousarray(np.asarray(W_qkv, dtype=np.float32).T)
    wqh = wq.astype(NP8)
    wql16 = (16.0 * (wq - wqh.astype(np.float32))).astype(NP8)
    wp = np.ascontiguousarray(np.asarray(W_proj, dtype=np.float32).T)
    wph = wp.astype(NP8)
    wpl16 = (16.0 * (wp - wph.astype(np.float32))).astype(NP8)
    wbias = (16.0 * np.asarray(b_proj, dtype=np.float32)).reshape(1, C).astype(
        ml_dtypes.bfloat16
    )
    xhi_i, xlo_i, xh16_i = ilv_x(xhi), ilv_x(xlo), ilv_x(xh16)
    in_maps = [
        {
            "xhi": xhi_i[NB * i : NB * (i + 1)],
            "xlo": xlo_i[NB * i : NB * (i + 1)],
            "xh16": xh16_i[NB * i : NB * (i + 1)],
            "wqh": _ilv_k(wqh),
            "wql": _ilv_k(wql16),
            "wph": _ilv_k(wph),
            "wpl": _ilv_k(wpl16),
            "wbias": wbias,
        }
        for i in range(N_CORES)
    ]
    res = run_bass_kernel_spmd(nc, in_maps, core_ids=list(range(N_CORES)))
    out = np.concatenate(
        [res.results[i]["out"].astype(np.float32) for i in range(N_CORES)], axis=0
    )
    return (out / 16.0).astype(np.float32)


if __name__ == "__main__":
    rng = np.random.default_rng(0)
    X = rng.standard_normal((B, N, C), dtype=np.float32)
    W_qkv = rng.standard_normal((3 * C, C), dtype=np.float32) * C**-0.5
    W_proj = rng.standard_normal((C, C), dtype=np.float32) * C**-0.5
    b_proj = np.zeros(C, dtype=np.float32)
    out = kernel(X, W_qkv, W_proj, b_proj, 1)
    print(out.shape, out.dtype)


# revision 38
# speedup vs baseline: 1.0571x; 1.0228x over previous
"""Trainium2 Bass kernel for two-stage pooled-query attention.

Problem (hardcoded):
    B=32, N=577, C=1024, H=16 heads, d=64, pooled queries 8x8 (3x3 mean over
    24x24 grid of non-cls tokens).
    qkv = X @ W_qkv.T ; pool Xq -> Qp ; s1 = softmax(Qp*s @ K^T) @ V ;
    s2 = softmax(Xq*s @ Qp^T) @ s1 ; out = s2 @ W_proj.T + b_proj

Strategy: pure data-parallel over batch across 8 NeuronCores (4 batches per
core, no collectives).

The two big GEMMs (QKV and output projection, ~75% of matmul cycles) run as
3-term fp8e4 DoubleRow matmuls: A@B ~ Ahi@Bhi + Alo@Bhi + (Ahi/16)@(16*Blo),
with K-chunk pairs packed into each DoubleRow instruction (effective K=256
per instruction at 0.5 cycles/row -> 4x bf16 throughput per K=128 product,
so the 3-term scheme runs at 0.75x bf16 cost with ~bf16 accuracy). X and the
weights arrive pre-split/interleaved from the host (layout prep); the
attention output is split on-chip during the PSUM evict, pre-scaled x16 (via
a 1/16 ones-column in V that scales Qd) so the residual stays out of fp8's
subnormal-flush range; the host divides the output by 16.

Attention middle stays bf16 (contractions are K<=128 there, DoubleRow does
not apply). Stage 1 is computed transposed relative to the baseline:
scores E1T[n, 2q] = exp(K^T Qp) directly (no PE transpose of A1), and the
softmax denominator comes from the ones-column in V, so Qd = E1T^T @ [V | 1]
yields numerator and denominator in one accumulation.

The whole thing is software-pipelined across batches: batch b+1's QKV/V
chunk-groups are emitted as PE filler interleaved into batch b's attention
middle (qkT/V/X are double-buffered), so the PE never drains during the
dependency-heavy softmax stretches.
"""

import os
import sys

import numpy as np

sys.path.insert(0, "/opt/trn_rl_repo")

import ml_dtypes  # noqa: E402

import concourse.tile as tile  # noqa: E402
from concourse import bacc, mybir  # noqa: E402
from concourse.bass_utils import run_bass_kernel_spmd  # noqa: E402
from concourse.masks import make_identity  # noqa: E402

B, N, C = 32, 577, 1024
H, D = 16, 64
SCALE = D ** -0.5
N_CORES = 8
NB = B // N_CORES  # batches per core

BF16 = mybir.dt.bfloat16
FP8 = mybir.dt.float8e4
F32 = mybir.dt.float32
DR = mybir.MatmulPerfMode.DoubleRow
NP8 = ml_dtypes.float8_e4m3

# token chunks of 577 = 4*128 + 65
TOK = [(0, 128), (128, 128), (256, 128), (384, 128), (512, 65)]
# free-dim chunks of 577 for wide matmuls / psum banks
NF = [(0, 320), (320, 257)]
EXP = mybir.ActivationFunctionType.Exp
SUB = mybir.AluOpType.subtract


def build_program(nb: int = NB):
    nc = bacc.Bacc("TRN2", target_bir_lowering=False, debug=False)

    # X pre-transposed [c, n], zero-padded to 640 tokens, split hi/lo/hi16
    # fp8 and K-pair interleaved: [pc, part, plane, tok].
    xhi_d = nc.dram_tensor("xhi", [nb, 128, 4, 2, 640], FP8, kind="ExternalInput")
    xlo_d = nc.dram_tensor("xlo", [nb, 128, 4, 2, 640], FP8, kind="ExternalInput")
    xh16_d = nc.dram_tensor("xh16", [nb, 128, 4, 2, 640], FP8, kind="ExternalInput")
    wqh_d = nc.dram_tensor("wqh", [128, 4, 2, 3 * C], FP8, kind="ExternalInput")
    wql_d = nc.dram_tensor("wql", [128, 4, 2, 3 * C], FP8, kind="ExternalInput")
    wph_d = nc.dram_tensor("wph", [128, 4, 2, C], FP8, kind="ExternalInput")
    wpl_d = nc.dram_tensor("wpl", [128, 4, 2, C], FP8, kind="ExternalInput")
    wbias_d = nc.dram_tensor("wbias", [1, C], BF16, kind="ExternalInput")
    out_d = nc.dram_tensor("out", [nb, N, C], BF16, kind="ExternalOutput")

    with tile.TileContext(nc) as tc:
        const_pool = tc.alloc_tile_pool(name="const", bufs=1)
        w_pool = tc.alloc_tile_pool(name="w", bufs=1)
        sb = tc.alloc_tile_pool(name="sb", bufs=2)
        ps_big = tc.alloc_tile_pool(name="ps_big", bufs=5, space="PSUM")
        ps_small = tc.alloc_tile_pool(name="ps_small", bufs=3, space="PSUM")

        ident = const_pool.tile([128, 128], BF16, tag="ident")
        make_identity(nc, ident[:])
        ones = const_pool.tile([1, 128], BF16, tag="ones")
        nc.gpsimd.memset(ones[:], 1.0)

        def dma_x_grp(b, grp):
            """DMA one of the three X tensors (0=hi, 1=lo, 2=hi16)."""
            tag, dram = [("xh", xhi_d), ("xl", xlo_d), ("x16", xh16_d)][grp]
            t = sb.tile([128, 4, 2, 640], FP8, tag=tag, bufs=2, name=tag)
            nc.sync.dma_start(t[:], dram[b])
            return t

        def dma_x(b):
            return tuple(dma_x_grp(b, g) for g in range(3))

        # weights stream in 512-col chunks interleaved with batch-0's X in
        # the order the QKV gemm consumes them (term-major: hi*hi first).
        WQH = w_pool.tile([128, 4, 2, 3 * C], FP8, tag="wqh", name="wqh")
        # WQL holds only the V columns: the q/k channels use the 2-term
        # product, so the lo-correction weights for them are never read
        WQL = w_pool.tile([128, 4, 2, C], FP8, tag="wql", name="wql")

        def dma_wq(which, blk):
            cs = slice(512 * blk, 512 * (blk + 1))
            if which == 0:
                nc.sync.dma_start(WQH[:, :, :, cs], wqh_d[:, :, :, cs])
            else:
                nc.sync.dma_start(
                    WQL[:, :, :, cs],
                    wql_d[:, :, :, 2048 + 512 * blk : 2048 + 512 * (blk + 1)],
                )

        repeat = int(os.environ.get("KERNEL_REPEAT", "1"))
        order = [bb for _ in range(repeat) for bb in range(nb)]

        def dma_x_grp_split(b, grp):
            tag, dram = [("xh", xhi_d), ("xl", xlo_d), ("x16", xh16_d)][grp]
            t = sb.tile([128, 4, 2, 640], FP8, tag=tag, bufs=2, name=tag)
            nc.sync.dma_start(t[:, 0:2], dram[b, :, 0:2])
            return t

        def finish_x_grp(t, b, grp):
            dram = [xhi_d, xlo_d, xh16_d][grp]
            nc.sync.dma_start(t[:, 2:4], dram[b, :, 2:4])

        XH0 = dma_x_grp_split(order[0], 0)
        nc.sync.dma_start(WQH[:, 0:2, :, 0:512], wqh_d[:, 0:2, :, 0:512])
        finish_x_grp(XH0, order[0], 0)
        nc.sync.dma_start(WQH[:, 2:4, :, 0:512], wqh_d[:, 2:4, :, 0:512])
        XL0 = dma_x_grp(order[0], 1)
        for blk in range(1, 4):
            dma_wq(0, blk)
        X160 = dma_x_grp(order[0], 2)
        X0 = (XH0, XL0, X160)
        dma_wq(0, 4)
        dma_wq(0, 5)
        dma_wq(1, 0)
        dma_wq(1, 1)
        # batch 1's X goes out before the proj weights: the pipelined filler
        # needs it early, the proj weights only by the first proj phase
        pre_x = {}
        if len(order) > 1:
            pre_x[1] = dma_x(order[1])
        WPH = w_pool.tile([128, 4, 2, C], FP8, tag="wph", name="wph")
        WPL = w_pool.tile([128, 4, 2, C], FP8, tag="wpl", name="wpl")
        nc.sync.dma_start(WPH[:], wph_d[:])
        nc.sync.dma_start(WPL[:], wpl_d[:])
        wb = w_pool.tile([1, C], BF16, tag="wb")
        nc.sync.dma_start(wb[:], wbias_d[:])

        # bias broadcast [128, 1024]; built lazily (first use is phase 8)
        bias = const_pool.tile([128, C], BF16, tag="bias")
        bias_built = [False]

        def build_bias():
            if bias_built[0]:
                return
            bias_built[0] = True
            for half in range(2):
                cs = slice(512 * half, 512 * (half + 1))
                bps = ps_big.tile([128, 512], F32, tag="pbig")
                nc.tensor.matmul(
                    bps[:], ones[0:1, :], wb[0:1, cs], start=True, stop=True
                )
                nc.any.tensor_copy(bias[:, cs], bps[:])

        # ------------------------------------------------------------------
        # per-batch state + emitters
        # ------------------------------------------------------------------
        class St:
            pass

        def make_state(b, X3):
            S = St()
            S.b = b
            S.XH, S.XL, S.X16 = X3
            S.qkT = [None] * 16
            S.QpBD = [None] * 8
            S.V = [None] * 5
            S.QdBD = [None] * 8
            S.A2n = [[None, None] for _ in range(5)]
            S.s1e = {}
            S.a2t = {}
            S.OTH = [sb.tile([128, 2, 640], FP8, tag=f"oth{pq}", bufs=1,
                             name=f"oth{pq}") for pq in range(4)]
            S.OTL = [sb.tile([128, 2, 640], FP8, tag=f"otl{pq}", bufs=1,
                             name=f"otl{pq}") for pq in range(4)]
            S.OT16 = [sb.tile([128, 2, 640], FP8, tag=f"ot16{pq}", bufs=1,
                              name=f"ot16{pq}") for pq in range(4)]
            return S

        def pool_chunk(S, j):
            qsum = sb.tile([128, 64], F32, tag="qsum", bufs=3)
            view = S.qkT[j][:, 0:576].rearrange(
                "p (pr dr pc dc) -> p pr pc dr dc", pr=8, dr=3, pc=8, dc=3
            )
            nc.vector.reduce_sum(qsum[:], view, axis=mybir.AxisListType.XY)
            qp = sb.tile([128, 128], BF16, tag=f"qp{j}", bufs=2, name=f"qp{j}")
            nc.gpsimd.memset(qp[:], 0.0)
            nc.gpsimd.tensor_scalar_mul(qp[0:64, 0:64], qsum[0:64, :], SCALE / 9.0)
            nc.gpsimd.tensor_scalar_mul(
                qp[64:128, 64:128], qsum[64:128, :], SCALE / 9.0)
            S.QpBD[j] = qp

        def qk_chunk(S, cc):
            qt = sb.tile([128, 640], BF16, tag=f"qkt{cc}", bufs=2,
                         name=f"qkt{cc}")
            ccs = slice(128 * cc, 128 * (cc + 1))
            for ci, (n0, nw) in enumerate(NF):
                ps = ps_big.tile([128, 512], F32, tag="pbig")
                k = 0
                # q/k channels use the cheapest 1-term fp8 product: the
                # quantization error only perturbs softmax logits a few
                # percent, which the attention normalization damps to well
                # under the error budget. V keeps all 3 terms.
                for pc in range(4):
                    nc.tensor.matmul(
                        ps[:, 0:nw],
                        WQH[:, pc, :, ccs],
                        S.XH[:, pc, :, n0 : n0 + nw],
                        start=(pc == 0),
                        stop=(pc == 3),
                        perf_mode=DR,
                    )
                if (cc + ci) % 2 == 0:
                    nc.vector.tensor_copy(qt[:, n0 : n0 + nw], ps[:, 0:nw])
                else:
                    nc.scalar.copy(qt[:, n0 : n0 + nw], ps[:, 0:nw])
            nc.gpsimd.memset(qt[:, 577:640], 0.0)
            S.qkT[cc] = qt
            if cc < 8:
                pool_chunk(S, cc)

        def v_chunk(S, t):
            toff, rows = TOK[t]
            vt = sb.tile([128, 8, 129], BF16, tag=f"v{t}", bufs=2, name=f"v{t}")
            lo = toff if rows == 128 else 512
            for h2 in range(2):
                ps = ps_big.tile([128, 512], F32, tag="pbig")
                k = 0
                for (wt, off, xt) in (
                    (WQH, 2048, S.XH), (WQH, 2048, S.XL), (WQL, 0, S.X16)
                ):
                    for pc in range(4):
                        nc.tensor.matmul(
                            ps[:],
                            xt[:, pc, :, lo : lo + 128],
                            wt[:, pc, :, off + 512 * h2 : off + 512 * (h2 + 1)],
                            start=(k == 0),
                            stop=(k == 11),
                            perf_mode=DR,
                        )
                        k += 1
                nc.any.tensor_copy(
                    vt[:, 4 * h2 : 4 * (h2 + 1), 0:128],
                    ps[:].rearrange("p (a b) -> p a b", a=4),
                )
            # 1/16 makes the s1 denominator den/16, so Qd evicts as 16*Qd:
            # the out_mm psum then sits in a healthy fp8 range for the
            # hi/lo split (host divides the output by 16).
            nc.gpsimd.memset(vt[:, :, 128:129], 1.0 / 16.0)
            S.V[t] = vt

        def qkv_work(S):
            w = [(lambda S=S, cc=cc: qk_chunk(S, cc)) for cc in range(16)]
            w += [(lambda S=S, t=t: v_chunk(S, t)) for t in range(5)]
            return w

        def s1_score(S, pp):
            e1t = sb.tile([128, 5, 128], BF16, tag="e1t", bufs=2)
            # zero the tail-chunk pad rows; the exp below rewrites row 64
            nc.gpsimd.memset(e1t[64:128, 4, :], 0.0)
            psa = ps_big.tile([128, 512], F32, tag="pbig")
            for t in range(4):
                nc.tensor.matmul(
                    psa[:, 128 * t : 128 * (t + 1)],
                    S.qkT[8 + pp][:, 128 * t : 128 * (t + 1)],
                    S.QpBD[pp][:],
                    start=True,
                    stop=True,
                )
            psb = ps_small.tile([128, 132], F32, tag="psmall")
            nc.tensor.matmul(
                psb[0:65, 0:128],
                S.qkT[8 + pp][:, 512:577],
                S.QpBD[pp][:],
                start=True,
                stop=True,
            )
            nc.scalar.activation(
                e1t[:, 0:4, :].rearrange("p a b -> p (a b)"), psa[:], EXP
            )
            nc.scalar.activation(e1t[0:65, 4, :], psb[0:65, 0:128], EXP)
            S.s1e[pp] = e1t

        def s1_qd(S, pp):
            e1t = S.s1e.pop(pp)
            psq = ps_small.tile([128, 132], F32, tag="psmall")
            for t in range(5):
                nc.tensor.matmul(
                    psq[:, 0:129],
                    e1t[:, t, :],
                    S.V[t][:, pp, :],
                    start=(t == 0),
                    stop=(t == 4),
                )
            r1 = sb.tile([128, 1], F32, tag="r1", bufs=4)
            nc.vector.reciprocal(r1[:], psq[:, 128:129])
            qd = sb.tile([128, 128], BF16, tag=f"qd{pp}", bufs=2, name=f"qd{pp}")
            nc.gpsimd.memset(qd[:], 0.0)
            nc.vector.tensor_scalar_mul(
                qd[0:64, 0:64], psq[0:64, 0:64], r1[0:64, 0:1]
            )
            nc.vector.tensor_scalar_mul(
                qd[64:128, 64:128], psq[64:128, 64:128], r1[64:128, 0:1]
            )
            S.QdBD[pp] = qd

        def s2_tok(S, oc, t):
            toff, rows = TOK[t]
            lo = toff if rows == 128 else 512
            ps = ps_big.tile([128, 512], F32, tag="pbig")
            for pz in range(4):
                pp = 4 * oc + pz
                nc.tensor.matmul(
                    ps[:, 128 * pz : 128 * (pz + 1)],
                    S.qkT[pp][:, lo : lo + 128],
                    S.QpBD[pp][:],
                    start=True,
                    stop=True,
                )
            s2e = sb.tile([128, 512], BF16, tag="s2e", bufs=4)
            nc.scalar.activation(s2e[0:rows, :], ps[0:rows, :], EXP)
            s2s = sb.tile([128, 8], F32, tag="s2s", bufs=4)
            nc.vector.reduce_sum(
                s2s[0:rows, :],
                s2e[0:rows, :].rearrange("p (h q) -> p h q", q=64),
                axis=mybir.AxisListType.X,
            )
            r2 = sb.tile([128, 8], F32, tag="r2", bufs=4)
            nc.vector.reciprocal(r2[0:rows, :], s2s[0:rows, :])
            a2 = sb.tile([128, 512], BF16, tag=f"a2n{t}_{oc}", bufs=1,
                         name=f"a2n{t}_{oc}")
            if rows < 128:
                # pad rows must be zero: the transpose DMA moves all 128 rows
                nc.gpsimd.memset(a2[64:128, :], 0.0)
            for pz in range(4):
                eng = nc.vector if pz == 0 else nc.gpsimd
                zs = slice(128 * pz, 128 * (pz + 1))
                eng.tensor_tensor(
                    a2[0:rows, zs].rearrange("p (h q) -> p h q", q=64),
                    s2e[0:rows, zs].rearrange("p (h q) -> p h q", q=64),
                    r2[0:rows, 2 * pz : 2 * pz + 2]
                    .unsqueeze(2)
                    .broadcast_to((rows, 2, 64)),
                    op=mybir.AluOpType.mult,
                )
            S.A2n[t][oc] = a2

        def a2_transpose(S, pp):
            oc, sl = pp // 4, 128 * (pp % 4)
            pa = ps_big.tile([128, 512], BF16, tag="pbig")
            for t in range(4):
                nc.tensor.transpose(
                    pa[:, 128 * t : 128 * (t + 1)],
                    S.A2n[t][oc][:, sl : sl + 128],
                    ident[:],
                )
            pb = ps_small.tile([128, 132], BF16, tag="psmall")
            # tail pad rows of A2n are zeroed, so the full block transposes
            # cleanly and a2t needs no pad memset
            nc.tensor.transpose(
                pb[:, 0:128], S.A2n[4][oc][:, sl : sl + 128], ident[:]
            )
            a2t = sb.tile([128, 640], BF16, tag="a2t", bufs=3)
            nc.any.tensor_copy(a2t[:, 0:512], pa[:])
            nc.any.tensor_copy(a2t[:, 512:640], pb[:, 0:128])
            S.a2t[pp] = a2t

        def out_mm(S, pp):
            a2t = S.a2t.pop(pp)
            pq, i = pp // 2, pp % 2
            oa = ps_big.tile([128, 512], F32, tag="pbig")
            ob = ps_small.tile([128, 132], F32, tag="psmall")
            nc.tensor.matmul(
                oa[:], S.QdBD[pp][:], a2t[:, 0:512], start=True, stop=True
            )
            nc.tensor.matmul(
                ob[:, 0:128], S.QdBD[pp][:], a2t[:, 512:640], start=True,
                stop=True,
            )
            nc.scalar.copy(S.OTH[pq][:, i, 0:512], oa[:])
            nc.scalar.copy(S.OTH[pq][:, i, 512:640], ob[:, 0:128])
            nc.vector.tensor_tensor(
                S.OTL[pq][:, i, 0:512], oa[:], S.OTH[pq][:, i, 0:512], op=SUB
            )
            nc.vector.tensor_tensor(
                S.OTL[pq][:, i, 512:640], ob[:, 0:128],
                S.OTH[pq][:, i, 512:640], op=SUB,
            )
            nc.gpsimd.tensor_scalar_mul(
                S.OT16[pq][:, i, :], S.OTH[pq][:, i, :], 1.0 / 16.0
            )

        def proj_chunk(S, t):
            toff, rows = TOK[t]
            lo = toff if rows == 128 else 512
            y = sb.tile([128, 1024], BF16, tag="y", bufs=2)
            for half in range(2):
                cs = slice(512 * half, 512 * (half + 1))
                ps = ps_big.tile([128, 512], F32, tag="pbig")
                k = 0
                for pq in range(4):
                    for (ot, wt) in ((S.OTH, WPH), (S.OTL, WPH), (S.OT16, WPL)):
                        nc.tensor.matmul(
                            ps[:],
                            ot[pq][:, :, lo : lo + 128],
                            wt[:, pq, :, cs],
                            start=(k == 0),
                            stop=(k == 11),
                            perf_mode=DR,
                        )
                        k += 1
                nc.vector.tensor_add(y[0:rows, cs], ps[0:rows, :],
                                     bias[0:rows, cs])
            nc.sync.dma_start(out_d[S.b, toff : toff + rows, :], y[0:rows, :])

        # ------------------------------------------------------------------
        # software-pipelined emission: batch b's attention middle is
        # interleaved with batch b+1's QKV/V chunk groups as PE filler.
        # ------------------------------------------------------------------
        S = make_state(order[0], X0)
        for w in qkv_work(S):
            w()

        for bi, b in enumerate(order):
            filler = []
            if bi + 1 < len(order):
                xn = pre_x.pop(bi + 1, None) or dma_x(order[bi + 1])
                Sn = make_state(order[bi + 1], xn)
                filler = qkv_work(Sn)
            fi = [0]

            def step(n=1):
                for _ in range(n):
                    if fi[0] < len(filler):
                        filler[fi[0]]()
                        fi[0] += 1

            # stage 1, with stage-2 chunks spread across the pair loop and
            # transpose-DMAs issued as soon as each octet's A2n completes
            for pp in range(8):
                s1_score(S, pp)
                if pp > 0:
                    s1_qd(S, pp - 1)
                if 1 <= pp <= 5:
                    s2_tok(S, 0, pp - 1)
                if 2 <= pp <= 6:
                    s2_tok(S, 1, pp - 2)
                step()
            s1_qd(S, 7)

            # transposes + outT matmuls (2-pair lag) + fp8 split evicts
            a2_transpose(S, 0)
            a2_transpose(S, 1)
            step()
            for pp in range(2, 8):
                a2_transpose(S, pp)
                out_mm(S, pp - 2)
                step()
            out_mm(S, 6)
            out_mm(S, 7)
            step()

            # output projection
            build_bias()
            for t in range(5):
                proj_chunk(S, t)
                step()
            step(len(filler))

            if filler:
                S = Sn

        for p in (ps_small, ps_big, sb, w_pool, const_pool):
            p.release()

    nc.compile()
    return nc


_NC_CACHE = {}


def _get_nc(nb: int = NB):
    if nb not in _NC_CACHE:
        _NC_CACHE[nb] = build_program(nb)
    return _NC_CACHE[nb]


def _ilv_k(a):
    """[K, F] -> [128, K//256, 2, F], partition-first with K-chunk pairs
    (2j, 2j+1) in the two DoubleRow planes."""
    Kd, F = a.shape
    return np.ascontiguousarray(a.reshape(Kd // 256, 2, 128, F).transpose(2, 0, 1, 3))


def kernel(X, W_qkv, W_proj, b_proj, layer_idx=None):
    assert X.shape == (B, N, C)
    nc = _get_nc(NB)
    xt = np.zeros((B, C, 640), dtype=np.float32)
    xt[:, :, :N] = np.asarray(X, dtype=np.float32).transpose(0, 2, 1)
    xhi = xt.astype(NP8)
    xhf = xhi.astype(np.float32)
    xlo = (xt - xhf).astype(NP8)
    xh16 = (xhf / 16.0).astype(NP8)

    def ilv_x(a):
        # [B, 1024, 640] -> [B, 128, 4, 2, 640]
        return np.ascontiguousarray(
            a.reshape(B, 4, 2, 128, 640).transpose(0, 3, 1, 2, 4)
        )

    wq = np.ascontiguousarray(np.asarray(W_qkv, dtype=np.float32).T)
    wqh = wq.astype(NP8)
    wql16 = (16.0 * (wq - wqh.astype(np.float32))).astype(NP8)
    wp = np.ascontiguousarray(np.asarray(W_proj, dtype=np.float32).T)
    wph = wp.astype(NP8)
    wpl16 = (16.0 * (wp - wph.astype(np.float32))).astype(NP8)
    wbias = (16.0 * np.asarray(b_proj, dtype=np.float32)).reshape(1, C).astype(
        ml_dtypes.bfloat16
    )
    xhi_i, xlo_i, xh16_i = ilv_x(xhi), ilv_x(xlo), ilv_x(xh16)
    in_maps = [
        {
            "xhi": xhi_i[NB * i : NB * (i + 1)],
            "xlo": xlo_i[NB * i : NB * (i + 1)],
            "xh16": xh16_i[NB * i : NB * (i + 1)],
            "wqh": _ilv_k(wqh),
            "wql": _ilv_k(wql16),
            "wph": _ilv_k(wph),
            "wpl": _ilv_k(wpl16),
            "wbias": wbias,
        }
        for i in range(N_CORES)
    ]
    res = run_bass_kernel_spmd(nc, in_maps, core_ids=list(range(N_CORES)))
    out = np.concatenate(
        [res.results[i]["out"].astype(np.float32) for i in range(N_CORES)], axis=0
    )
    return (out / 16.0).astype(np.float32)


if __name__ == "__main__":
    rng = np.random.default_rng(0)
    X = rng.standard_normal((B, N, C), dtype=np.float32)
    W_qkv = rng.standard_normal((3 * C, C), dtype=np.float32) * C**-0.5
    W_proj = rng.standard_normal((C, C), dtype=np.float32) * C**-0.5
    b_proj = np.zeros(C, dtype=np.float32)
    out = kernel(X, W_qkv, W_proj, b_proj, 1)
    print(out.shape, out.dtype)
